# revision 1
# baseline (speedup 1.0000x reference)
"""CSPN (convolutional spatial propagation network) kernel for Trainium2.

Reference computation (per batch image, 512x512, fp32):
  aff    = conv3x3(x, W_aff, SAME) + b_aff          # 8 channels
  a      = aff / sum_c |aff_c| ; s = sum_c a_c
  kernel = concat([1 - s, a])                       # 9 channels
  24 iterations:  x <- sum_k kernel_k * shift_{OFFS[k]}(x)   (zero padded)

Sharding: data-parallel over batch, one image per NeuronCore (8 cores).

Per-core design (everything SBUF resident, all fp32):
  * x state in two ping/pong buffers, layout [128 partitions, 6*514]:
    partition p holds image rows 4p..4p+3 in row slots 1..4 plus halo row
    slots 0 (row 4p-1) and 5 (row 4p+4); each row slot is 514 wide with a
    zero pad column each side.  Halo slots are refreshed after every
    iteration with two partition-shifted SBUF->SBUF DMAs, overlapped with
    the next iteration's centre-row products.
  * 9-channel diffusion kernel at [c][r][j] (free offset c*2048 + r*512 + j).
  * affinity conv on the vector engine: one fused scalar_tensor_tensor MAC
    per (channel, tap) — aff += W[c,a,b] * shifted x — with the 3x3 weights
    as per-partition scalars broadcast via a K=1 matmul through PSUM.
  * kernel generation: abs-reduce over channels, 2-ULP reciprocal, scale.
  * diffusion: 18 vector-engine tensor_tensor ops per iteration
    (9 products + 9 accumulate adds; final add split for halo overlap).
"""

import numpy as np

H = 512
W = 512
B = 8
ITER = 24
# itertools.product([0,1,-1], repeat=2) order (matches reference OFFS)
OFFS = [(i, j) for i in (0, 1, -1) for j in (0, 1, -1)]

WP = W + 2            # padded row width
NSLOT = 6             # row slots per partition (1 halo + 4 + 1 halo)
RJ = 4 * W            # 2048 free elems per channel plane per partition
AFF_CH = 8

_PROGRAM = None


def _build_program():
    import concourse.mybir as mybir
    from concourse import bacc, tile

    f32 = mybir.dt.float32
    mult = mybir.AluOpType.mult
    add = mybir.AluOpType.add

    nc = bacc.Bacc("TRN2", target_bir_lowering=False, debug=False, name="cspn")

    x_d = nc.dram_tensor("x", [H, W], f32, kind="ExternalInput")
    w_d = nc.dram_tensor("w_aff", [AFF_CH * 9], f32, kind="ExternalInput")
    b_d = nc.dram_tensor("b_aff", [AFF_CH], f32, kind="ExternalInput")
    out_d = nc.dram_tensor("out", [H, W], f32, kind="ExternalOutput")

    with tile.TileContext(nc) as tc:
        with (
            tc.tile_pool(name="state", bufs=1) as sp,
            tc.tile_pool(name="psum", bufs=1, space="PSUM") as pp,
        ):
            xb0 = sp.tile([128, NSLOT * WP], f32, tag="xb0")
            xb1 = sp.tile([128, NSLOT * WP], f32, tag="xb1")
            kern = sp.tile([128, 9 * RJ], f32, tag="kern")
            acc = sp.tile([128, RJ], f32, tag="acc")
            pr = sp.tile([128, RJ], f32, tag="pr")
            sums = sp.tile([128, RJ], f32, tag="sums")
            recip = sp.tile([128, RJ], f32, tag="recip")
            wbc = sp.tile([128, 80], f32, tag="wbc")
            ones = sp.tile([1, 128], f32, tag="ones")

            xv0 = xb0[:].rearrange("p (s w) -> p s w", w=WP)
            xv1 = xb1[:].rearrange("p (s w) -> p s w", w=WP)
            xviews = [xv0, xv1]

            # ---------------- init / loads ----------------
            nc.vector.memset(xb0[:], 0.0)
            nc.vector.memset(xb1[:], 0.0)
            nc.gpsimd.memset(ones[:], 1.0)

            nc.sync.dma_start(
                out=xv0[:, 1:5, 1 : 1 + W],
                in_=x_d.rearrange("(p r) w -> p r w", p=128),
            )
            # initial halo rows for xb0
            nc.sync.dma_start(out=xv0[1:128, 0, 1:513], in_=xv0[0:127, 4, 1:513])
            nc.sync.dma_start(out=xv0[0:127, 5, 1:513], in_=xv0[1:128, 1, 1:513])

            # w/b broadcast to all partitions via a K=1 matmul through PSUM
            nc.sync.dma_start(out=wbc[:1, :72], in_=w_d[None, :])
            nc.sync.dma_start(out=wbc[:1, 72:80], in_=b_d[None, :])
            pw = pp.tile([128, 80], f32, tag="wps")
            nc.tensor.matmul(pw[:, :], ones[:1, :], wbc[:1, :80],
                             start=True, stop=True)
            nc.vector.tensor_copy(out=wbc[:, :80], in_=pw[:, :])

            # ---------------- affinity conv (DVE, fused MAC per tap) -------
            # aff channel c lives in kern channel 1+c
            # aff_c[4p+r, j] = b_c + sum_{a,b} W[c,a,b] * x[4p+r+a-1, j+b-1]
            #   x row 4p+r+a-1 -> slot r+a ; col j+b-1 -> stored col j+b
            for c in range(AFF_CH):
                av = kern[:, (1 + c) * RJ : (2 + c) * RJ].rearrange(
                    "p (r j) -> p r j", j=W
                )
                for a in range(3):
                    for b3 in range(3):
                        xsh = xv0[:, a : a + 4, b3 : b3 + W]
                        wsc = wbc[:, c * 9 + a * 3 + b3 : c * 9 + a * 3 + b3 + 1]
                        if a == 0 and b3 == 0:
                            nc.vector.tensor_scalar(
                                out=av, in0=xsh, scalar1=wsc,
                                scalar2=wbc[:, 72 + c : 73 + c],
                                op0=mult, op1=add,
                            )
                        else:
                            nc.vector.scalar_tensor_tensor(
                                out=av, in0=xsh, scalar=wsc, in1=av,
                                op0=mult, op1=add,
                            )

            # ---------------- kernel generation ----------------
            aff = kern[:, RJ : 9 * RJ]
            affv = aff.rearrange("p (c rj) -> p rj c", c=AFF_CH)
            nc.vector.tensor_reduce(
                out=sums[:], in_=affv, axis=mybir.AxisListType.X,
                op=add, apply_absolute_value=True,
            )
            nc.vector.reciprocal_approx_accurate(recip[:], sums[:], scratch=acc[:])
            affc = aff.rearrange("p (c rj) -> p c rj", c=AFF_CH)
            rb = recip[:].unsqueeze(1).broadcast_to([128, AFF_CH, RJ])
            nc.vector.tensor_tensor(out=affc, in0=affc, in1=rb, op=mult)
            nc.vector.tensor_reduce(
                out=sums[:], in_=affv, axis=mybir.AxisListType.X, op=add,
            )
            # kern0 = 1 - s
            nc.vector.tensor_scalar(
                out=kern[:, 0:RJ], in0=sums[:], scalar1=-1.0, scalar2=1.0,
                op0=mult, op1=add,
            )

            # ---------------- diffusion ----------------
            accv = acc[:].rearrange("p (r j) -> p r j", j=W)
            prv = pr[:].rearrange("p (r j) -> p r j", j=W)
            for it in range(ITER):
                cur = xviews[it % 2]
                nxt = xviews[(it + 1) % 2]
                for c, (oi, oj) in enumerate(OFFS):
                    kv = kern[:, c * RJ : (c + 1) * RJ].rearrange(
                        "p (r j) -> p r j", j=W
                    )
                    xsh = cur[:, 1 - oi : 5 - oi, 1 - oj : 513 - oj]
                    if c == 0:
                        nc.vector.tensor_tensor(out=accv, in0=kv, in1=xsh, op=mult)
                    elif c < 8:
                        nc.vector.tensor_tensor(out=prv, in0=kv, in1=xsh, op=mult)
                        nc.vector.tensor_tensor(out=accv, in0=accv, in1=prv, op=add)
                    else:
                        nc.vector.tensor_tensor(out=prv, in0=kv, in1=xsh, op=mult)
                        # edge rows (r=0,3) first so halo DMAs launch early
                        nc.vector.tensor_tensor(
                            out=nxt[:, 1:5:3, 1:513], in0=accv[:, 0:4:3, :],
                            in1=prv[:, 0:4:3, :], op=add,
                        )
                        nc.sync.dma_start(
                            out=nxt[1:128, 0, 1:513], in_=nxt[0:127, 4, 1:513]
                        )
                        nc.sync.dma_start(
                            out=nxt[0:127, 5, 1:513], in_=nxt[1:128, 1, 1:513]
                        )
                        nc.vector.tensor_tensor(
                            out=nxt[:, 2:4, 1:513], in0=accv[:, 1:3, :],
                            in1=prv[:, 1:3, :], op=add,
                        )

            nc.sync.dma_start(
                out=out_d.rearrange("(p r) w -> p r w", p=128),
                in_=xviews[ITER % 2][:, 1:5, 1:513],
            )

    nc.finalize()
    return nc


def _get_program():
    global _PROGRAM
    if _PROGRAM is None:
        _PROGRAM = _build_program()
    return _PROGRAM


def kernel(x, W_aff, b_aff):
    from concourse.bass_utils import run_bass_kernel_spmd

    nc = _get_program()
    x = np.ascontiguousarray(np.asarray(x, dtype=np.float32))
    w = np.ascontiguousarray(np.asarray(W_aff, dtype=np.float32)).reshape(AFF_CH * 9)
    b = np.ascontiguousarray(np.asarray(b_aff, dtype=np.float32))

    in_maps = [{"x": x[i, 0], "w_aff": w, "b_aff": b} for i in range(B)]
    res = run_bass_kernel_spmd(nc, in_maps, list(range(B))).results
    out = np.stack([res[i]["out"] for i in range(B)], axis=0)[:, None]
    return out.astype(np.float32)



# revision 16
# speedup vs baseline: 2.1412x; 2.1412x over previous
"""CSPN (convolutional spatial propagation network) kernel for Trainium2.

Reference computation (per batch image, 512x512, fp32):
  aff    = conv3x3(x, W_aff, SAME) + b_aff          # 8 channels
  a      = aff / sum_c |aff_c| ; s = sum_c a_c
  kernel = concat([1 - s, a])                       # 9 channels
  24 iterations:  x <- sum_k kernel_k * shift_{OFFS[k]}(x)   (zero padded)

Sharding: data-parallel over batch, one image per NeuronCore (8 cores).

Per-core design (all SBUF resident):
  * state in fp16; the 9-plane kernel is pre-scaled by 0.5 so every
    diffusion step halves the field (keeps fp16 in range); the final
    output is scaled back by 2^24 during the fp16->fp32 copy-out.
    Validated numerically: worst-case rel err ~6e-3 vs fp32 reference.
  * x state ping/pong [128 part, 6*514] fp16: partition p holds rows
    4p..4p+3 in slots 1..4, halo rows in slots 0/5, zero pad columns.
  * kernel planes [128, 9*2048] fp16, plane c=3a+b multiplies
    x[r+a-1, j+b-1] (plane order chosen so product access patterns are
    ascending-stride; ref channel (oi,oj) lands at plane (1-oi)*3+(1-oj)).
  * diffusion iteration, split by columns across two engines:
      - DVE (cols 0..CD-1): 3 fp16 tensor_tensor product ops (3 planes
        each via overlapping access patterns, 2x_1p mode) + in-place add
        pyramid + edge/center final adds.
      - Pool/GPSIMD (cols CD..511): same 3 product ops on its columns +
        two 9-way tensor_reduce ops (edge rows then center rows).
      - 2 SBUF->SBUF partition-shifted halo DMAs per iteration, launched
        after the edge-row results, consumed one product-op into the
        next iteration.
  * affinity conv: per channel, 9 tensor_scalar ops (x * w + b, 4x DVE
    mode, weights broadcast per-partition via a K=1 matmul) into a
    scratch plane-stack, then DVE add-pyramid / Pool tensor_reduce by
    column split; two scratch stacks so channel k+1's products overlap
    channel k's reduction.
  * kernel generation: abs-sum tensor_reduce (column-split DVE/Pool),
    fast-NR reciprocal, scale+halve on the scalar engine, fp16
    normalize, sum pyramid, center plane = 0.5 - s_half.
"""

import numpy as np

H = 512
W = 512
B = 8
ITER = 24
# itertools.product([0,1,-1], repeat=2) order (matches reference OFFS)
OFFS = [(i, j) for i in (0, 1, -1) for j in (0, 1, -1)]

WP = W + 2            # padded row width
NSLOT = 6             # row slots per partition (1 halo + 4 + 1 halo)
RJ = 4 * W            # 2048 elems per plane per partition
AFF_CH = 8


# ref aff channel m (kernel channel m+1, offset OFFS[m+1]) -> plane (1-oi)*3+(1-oj)
PLANE_OF = [(1 - oi) * 3 + (1 - oj) for (oi, oj) in OFFS[1:]]

_PROGRAM = None


def _build_program(iters=ITER, channels=AFF_CH):
    import concourse.mybir as mybir
    from concourse import bacc, tile
    from concourse.ap import AP

    f32 = mybir.dt.float32
    f16 = mybir.dt.float16
    mult = mybir.AluOpType.mult
    add = mybir.AluOpType.add
    Ax = mybir.AxisListType.X

    nc = bacc.Bacc("TRN2", target_bir_lowering=False, debug=False, name="cspn")

    x_d = nc.dram_tensor("x", [H, W], f32, kind="ExternalInput")
    sdn_d = nc.dram_tensor("sdn", [128, 128], f16, kind="ExternalInput")
    sup_d = nc.dram_tensor("sup", [128, 128], f16, kind="ExternalInput")
    w_d = nc.dram_tensor("w_aff", [AFF_CH * 9], f32, kind="ExternalInput")
    b_d = nc.dram_tensor("b_aff", [AFF_CH], f32, kind="ExternalInput")
    out_d = nc.dram_tensor("out", [H, W], f32, kind="ExternalOutput")

    with tile.TileContext(nc) as tc:
        with (
            nc.allow_low_precision(reason="fp16 scheme validated: rel err ~6e-3 vs 2e-2 budget"),
            tc.tile_pool(name="state", bufs=1) as sp,
            tc.tile_pool(name="psum", bufs=1, space="PSUM") as pp,
        ):
            xb0 = sp.tile([128, NSLOT * WP], f16, tag="xb0")
            xb1 = sp.tile([128, NSLOT * WP], f16, tag="xb1")
            kern = sp.tile([128, 9 * RJ], f16, tag="kern")
            tmpA = sp.tile([128, 9 * RJ], f16, tag="tmpA")
            tmpB = sp.tile([128, 9 * RJ], f16, tag="tmpB")
            aff = sp.tile([128, 9 * RJ], f16, tag="aff")  # 9 planes (4 unused)
            stage = sp.tile([128, RJ], f32, tag="stage")
            sums = sp.tile([128, RJ], f32, tag="sums")
            reciph = sp.tile([128, RJ], f16, tag="reciph")
            wbc = sp.tile([128, 80], f32, tag="wbc")
            ones = sp.tile([1, 128], f32, tag="ones")
            absmask = sp.tile([128, 1], mybir.dt.uint16, tag="absmask")
            sdn = sp.tile([128, 128], f16, tag="sdn")
            sup = sp.tile([128, 128], f16, tag="sup")
            psd = pp.tile([128, W], f32, tag="psd")
            psu = pp.tile([128, W], f32, tag="psu")

            xv0 = xb0[:].rearrange("p (s w) -> p s w", w=WP)
            xv1 = xb1[:].rearrange("p (s w) -> p s w", w=WP)
            xviews = [xv0, xv1]
            xtiles = [xb0, xb1]

            # ---------------- init / loads ----------------
            nc.vector.memset(xb0[:], 0.0)
            nc.gpsimd.memset(xb1[:], 0.0)
            nc.gpsimd.memset(ones[:], 1.0)

            nc.sync.dma_start(
                out=stage[:].rearrange("p (r j) -> p r j", j=W),
                in_=x_d.rearrange("(p r) w -> p r w", p=128),
            )
            nc.sync.dma_start(out=sdn[:], in_=sdn_d[:, :])
            nc.sync.dma_start(out=sup[:], in_=sup_d[:, :])
            # w/b broadcast to all partitions via a K=1 matmul through PSUM
            nc.sync.dma_start(out=wbc[:1, :72], in_=w_d[None, :])
            nc.sync.dma_start(out=wbc[:1, 72:80], in_=b_d[None, :])
            pw = pp.tile([128, 80], f32, tag="wps")
            nc.tensor.matmul(pw[:, :], ones[:1, :], wbc[:1, :80],
                             start=True, stop=True)
            nc.vector.tensor_copy(out=wbc[:, :80], in_=pw[:, :])

            # fp32 -> fp16 state (scalar engine), then initial halo rows
            nc.scalar.copy(out=xv0[:, 1:5, 1:1 + W],
                           in_=stage[:].rearrange("p (r j) -> p r j", j=W))

            def pe_halo(nxt):
                # halo rows via PE partition shift (PSUM) + Act copy-back;
                # boundary partitions get exact zeros from the shift matrices
                nc.tensor.matmul(psd[:, :], sdn[:, :], nxt[:, 4, 1:513],
                                 start=True, stop=True)
                nc.tensor.matmul(psu[:, :], sup[:, :], nxt[:, 1, 1:513],
                                 start=True, stop=True)
                nc.scalar.copy(out=nxt[:, 0, 1:513], in_=psd[:, :])
                nc.scalar.copy(out=nxt[:, 5, 1:513], in_=psu[:, :])

            pe_halo(xv0)

            # ---------------- affinity conv ----------------
            # per channel: 9 tensor_scalar products (fp16 4x) into tmp stack,
            # DVE pyramid on cols [0:CD_CONV), Pool 9-way reduce on the rest.
            affv = aff[:].rearrange("p (c rj) -> p c rj", c=9)
            tmps = [tmpA, tmpB]
            for m in range(channels):
                cp = PLANE_OF[m]
                tm = tmps[m % 2]
                tv = tm[:].rearrange("p (c r j) -> p c r j", c=9, j=W)
                for t in range(9):
                    a, b3 = divmod(t, 3)
                    xin = xv0[:, a:a + 4, b3:b3 + W]
                    wsc = wbc[:, 9 * m + t:9 * m + t + 1]
                    if t == 0:
                        nc.vector.tensor_scalar(
                            out=tv[:, 0], in0=xin, scalar1=wsc,
                            scalar2=wbc[:, 72 + m:73 + m], op0=mult, op1=add)
                    else:
                        nc.vector.tensor_scalar(
                            out=tv[:, t], in0=xin, scalar1=wsc, scalar2=None,
                            op0=mult)
                # add pyramid (DVE; Pool measured slower even for slices)
                tc2 = tm[:].rearrange("p (c rj) -> p c rj", c=9)
                nc.vector.tensor_tensor(out=tc2[:, 0:4], in0=tc2[:, 0:4],
                                        in1=tc2[:, 4:8], op=add)
                nc.vector.tensor_tensor(out=tc2[:, 0:2], in0=tc2[:, 0:2],
                                        in1=tc2[:, 2:4], op=add)
                nc.vector.tensor_tensor(out=tc2[:, 0], in0=tc2[:, 0],
                                        in1=tc2[:, 1], op=add)
                nc.vector.tensor_tensor(out=affv[:, cp], in0=tc2[:, 0],
                                        in1=tc2[:, 8], op=add)

            # ---------------- kernel generation ----------------
            # abs of the 8 aff planes (skip 4) into tmpB planes 0..7 via
            # sign-bit clear (uint16 bitwise_and, ts 4x), then fp16
            # add-pyramid (column-split) -> s_abs in tmpB plane 0
            u16 = mybir.dt.uint16
            band = mybir.AluOpType.bitwise_and
            tb = tmpB[:].rearrange("p (c rj) -> p c rj", c=9)
            tbj = tmpB[:].rearrange("p (c r j) -> p c r j", c=9, j=W)
            tb_u = tmpB[:].bitcast(u16).rearrange("p (c rj) -> p c rj", c=9)
            aff_u = aff[:].bitcast(u16).rearrange("p (c rj) -> p c rj", c=9)
            nc.gpsimd.memset(absmask[:], 0x7FFF)
            for i, cp in enumerate([0, 1, 2, 3, 5, 6, 7, 8]):
                nc.vector.tensor_scalar(out=tb_u[:, i], in0=aff_u[:, cp],
                                        scalar1=absmask[:], scalar2=None, op0=band)
            nc.vector.tensor_tensor(out=tb[:, 0:4], in0=tb[:, 0:4],
                                    in1=tb[:, 4:8], op=add)
            nc.vector.tensor_tensor(out=tb[:, 0:2], in0=tb[:, 0:2],
                                    in1=tb[:, 2:4], op=add)
            nc.vector.tensor_tensor(out=tb[:, 0], in0=tb[:, 0],
                                    in1=tb[:, 1], op=add)
            # abs-sum (fp16) -> fp32, fast-NR reciprocal, halve+convert on Act
            nc.vector.tensor_copy(out=sums[:], in_=tb[:, 0])
            nc.vector.reciprocal_approx_fast(stage[:], sums[:])
            nc.scalar.mul(out=reciph[:], in_=stage[:], mul=0.5)
            # kern planes = aff planes * reciph (fp16 2x), split DVE/Pool
            kv = kern[:].rearrange("p (c rj) -> p c rj", c=9)
            kvj = kern[:].rearrange("p (c r j) -> p c r j", c=9, j=W)
            rb = reciph[:].unsqueeze(1).broadcast_to([128, 4, RJ])
            nc.vector.tensor_tensor(out=kv[:, 0:4], in0=affv[:, 0:4], in1=rb, op=mult)
            nc.vector.tensor_tensor(out=kv[:, 5:9], in0=affv[:, 5:9], in1=rb, op=mult)
            # s_half tree: T1 = K[0:4]+K[5:9]; T2 = T1[0:2]+T1[2:4]; s = T2[0]+T2[1]
            tv = tmpA[:].rearrange("p (c rj) -> p c rj", c=9)
            nc.vector.tensor_tensor(out=tv[:, 0:4], in0=kv[:, 0:4], in1=kv[:, 5:9], op=add)
            nc.vector.tensor_tensor(out=tv[:, 0:2], in0=tv[:, 0:2], in1=tv[:, 2:4], op=add)
            nc.vector.tensor_tensor(out=tv[:, 0], in0=tv[:, 0], in1=tv[:, 1], op=add)
            # kern plane 4 = 0.5 - s_half
            nc.vector.tensor_scalar(out=kv[:, 4], in0=tv[:, 0], scalar1=-1.0,
                                    scalar2=0.5, op0=mult, op1=add)

            # ---------------- diffusion ----------------
            # DVE-only compute; halo rows travel across partitions through
            # the (otherwise idle) tensor engine + scalar-engine copy-back.
            # Product ops are split so halo-consuming rows come last, giving
            # the PE+Act chain a full product-phase of slack.
            prod = tmpA
            pv = prod[:].rearrange("p (c r j) -> p c r j", c=9, j=W)

            def products(curt, a, s0, s1):
                # prod planes 3a..3a+2 rows [s0:s1): overlapping-AP fp16 mult
                xap = curt[:]
                in1 = AP(xap.tensor, (a + s0) * WP,
                         [list(xap.ap[0]), [1, 3], [WP, s1 - s0], [1, W]])
                nc.vector.tensor_tensor(out=pv[:, 3 * a:3 * a + 3, s0:s1, :],
                                        in0=kvj[:, 3 * a:3 * a + 3, s0:s1, :],
                                        in1=in1, op=mult)

            for it in range(iters):
                curt = xtiles[it % 2]
                nxt = xviews[(it + 1) % 2]
                # halo-free rows first; halo-consuming strips last
                for (a, s0, s1) in ((1, 0, 4), (0, 1, 4), (2, 0, 3), (0, 0, 1), (2, 3, 4)):
                    products(curt, a, s0, s1)
                nc.vector.tensor_tensor(out=pv[:, 0:4], in0=pv[:, 0:4],
                                        in1=pv[:, 4:8], op=add)
                nc.vector.tensor_tensor(out=pv[:, 0:2], in0=pv[:, 0:2],
                                        in1=pv[:, 2:4], op=add)
                nc.vector.tensor_tensor(out=pv[:, 0], in0=pv[:, 0],
                                        in1=pv[:, 1], op=add)
                # edge rows (slots 1,4) first so the halo shift launches early
                nc.vector.tensor_tensor(
                    out=nxt[:, 1:5:3, 1:513], in0=pv[:, 0, 0:4:3, :],
                    in1=pv[:, 8, 0:4:3, :], op=add)
                if it + 1 < iters:
                    pe_halo(nxt)
                nc.vector.tensor_tensor(
                    out=nxt[:, 2:4, 1:513], in0=pv[:, 0, 1:3, :],
                    in1=pv[:, 8, 1:3, :], op=add)

            # ---------------- output: fp16 -> fp32 * 2^24 ----------------
            nc.scalar.mul(out=stage[:].rearrange("p (r j) -> p r j", j=W),
                          in_=xviews[iters % 2][:, 1:5, 1:513], mul=float(2.0 ** 24))
            nc.sync.dma_start(
                out=out_d.rearrange("(p r) w -> p r w", p=128),
                in_=stage[:].rearrange("p (r j) -> p r j", j=W),
            )

    nc.finalize()
    return nc


def _get_program():
    global _PROGRAM
    if _PROGRAM is None:
        _PROGRAM = _build_program()
    return _PROGRAM


def kernel(x, W_aff, b_aff):
    from concourse.bass_utils import run_bass_kernel_spmd

    nc = _get_program()
    x = np.ascontiguousarray(np.asarray(x, dtype=np.float32))
    w = np.ascontiguousarray(np.asarray(W_aff, dtype=np.float32)).reshape(AFF_CH * 9)
    b = np.ascontiguousarray(np.asarray(b_aff, dtype=np.float32))
    sdn = np.zeros((128, 128), np.float16)
    sdn[np.arange(127), np.arange(1, 128)] = 1
    sup = np.zeros((128, 128), np.float16)
    sup[np.arange(1, 128), np.arange(127)] = 1

    in_maps = [{"x": x[i, 0], "w_aff": w, "b_aff": b, "sdn": sdn, "sup": sup}
               for i in range(B)]
    res = run_bass_kernel_spmd(nc, in_maps, list(range(B))).results
    out = np.stack([res[i]["out"] for i in range(B)], axis=0)[:, None]
    return out.astype(np.float32)


# revision 19
# speedup vs baseline: 4.0423x; 1.8879x over previous
"""CSPN (convolutional spatial propagation network) kernel for Trainium2.

Reference computation (per batch image, 512x512, fp32):
  aff    = conv3x3(x, W_aff, SAME) + b_aff          # 8 channels
  a      = aff / sum_c |aff_c| ; s = sum_c a_c
  kernel = concat([1 - s, a])                       # 9 channels
  24 iterations:  x <- sum_k kernel_k * shift_{OFFS[k]}(x)   (zero padded)

Sharding: data-parallel over batch, one image per NeuronCore (8 cores).

Per-core design (all SBUF resident, all four engines in play):
  * state in fp16; the 9-plane kernel is pre-scaled by 0.5 so every
    diffusion step halves the field (keeps fp16 in range); the final
    output is scaled back by 2^24 during the fp16->fp32 copy-out.
    Measured rel err ~9.4e-3 vs fp32 reference (2e-2 budget).
  * x state ping/pong [128 part, 6*514] fp16: partition p holds rows
    4p..4p+3 in slots 1..4, halo rows in slots 0/5, zero pad columns.
  * kernel planes [128, 9*2048] fp16, plane c=3a+b multiplies
    x[r+a-1, j+b-1] (ref channel (oi,oj) lands at plane (1-oi)*3+(1-oj)).
  * diffusion iteration (engines pipelined per image row):
      - DVE: only the 9 shifted products (fp16 2x_1p mode, overlapping
        access patterns, one op per row x plane-group; halo-consuming
        rows ordered last),
      - PE: the 9-way summation as chains of accumulating matmuls with
        an identity stationary into PSUM (one chain per row, exact fp32),
      - Act: PSUM -> fp16 next-state copy per row,
      - halo rows travel across partitions via shifted-identity matmuls
        (pe_halo) + Act copy-back; boundary partitions get exact zeros.
        Measured ~6x cheaper than partition-shifted SBUF->SBUF DMAs.
      - GPSIMD/Pool measured far below its cost model on sliced ops
        (~2.2us fixed per op), so it only does init memsets.
  * affinity conv: per channel, 9 tensor_scalar products (x * w + b,
    4x DVE mode, weights broadcast per-partition via a K=1 matmul
    through PSUM) into a double-buffered scratch stack; PE accumulate
    chains + Act copies produce the aff plane while DVE starts the next
    channel.
  * kernel generation: |aff| via sign-bit clear (uint16 bitcast AND),
    fp16 abs-sum pyramid, fast-NR reciprocal, halve+convert on the
    scalar engine, fp16 normalize, sum pyramid, plane 4 = 0.5 - s_half.
"""

import numpy as np

H = 512
W = 512
B = 8
ITER = 24
# itertools.product([0,1,-1], repeat=2) order (matches reference OFFS)
OFFS = [(i, j) for i in (0, 1, -1) for j in (0, 1, -1)]

WP = W + 2            # padded row width
NSLOT = 6             # row slots per partition (1 halo + 4 + 1 halo)
RJ = 4 * W            # 2048 elems per plane per partition
AFF_CH = 8


# ref aff channel m (kernel channel m+1, offset OFFS[m+1]) -> plane (1-oi)*3+(1-oj)
PLANE_OF = [(1 - oi) * 3 + (1 - oj) for (oi, oj) in OFFS[1:]]

_PROGRAM = None


def _build_program(iters=ITER, channels=AFF_CH):
    import concourse.mybir as mybir
    from concourse import bacc, tile
    from concourse.ap import AP

    f32 = mybir.dt.float32
    f16 = mybir.dt.float16
    mult = mybir.AluOpType.mult
    add = mybir.AluOpType.add
    Ax = mybir.AxisListType.X

    nc = bacc.Bacc("TRN2", target_bir_lowering=False, debug=False, name="cspn")

    x_d = nc.dram_tensor("x", [H, W], f32, kind="ExternalInput")
    sdn_d = nc.dram_tensor("sdn", [128, 128], f16, kind="ExternalInput")
    sup_d = nc.dram_tensor("sup", [128, 128], f16, kind="ExternalInput")
    id_d = nc.dram_tensor("ident", [128, 128], f16, kind="ExternalInput")
    w_d = nc.dram_tensor("w_aff", [AFF_CH * 9], f32, kind="ExternalInput")
    b_d = nc.dram_tensor("b_aff", [AFF_CH], f32, kind="ExternalInput")
    out_d = nc.dram_tensor("out", [H, W], f32, kind="ExternalOutput")

    with tile.TileContext(nc) as tc:
        with (
            nc.allow_low_precision(reason="fp16 scheme validated: rel err ~6e-3 vs 2e-2 budget"),
            tc.tile_pool(name="state", bufs=1) as sp,
            tc.tile_pool(name="psum", bufs=1, space="PSUM") as pp,
        ):
            xb0 = sp.tile([128, NSLOT * WP], f16, tag="xb0")
            xb1 = sp.tile([128, NSLOT * WP], f16, tag="xb1")
            kern = sp.tile([128, 9 * RJ], f16, tag="kern")
            tmpA = sp.tile([128, 9 * RJ], f16, tag="tmpA")
            tmpB = sp.tile([128, 9 * RJ], f16, tag="tmpB")
            aff = sp.tile([128, 9 * RJ], f16, tag="aff")  # 9 planes (4 unused)
            stage = sp.tile([128, RJ], f32, tag="stage")
            sums = sp.tile([128, RJ], f32, tag="sums")
            reciph = sp.tile([128, RJ], f16, tag="reciph")
            wbc = sp.tile([128, 80], f32, tag="wbc")
            ones = sp.tile([1, 128], f32, tag="ones")
            absmask = sp.tile([128, 1], mybir.dt.uint16, tag="absmask")
            sdn = sp.tile([128, 128], f16, tag="sdn")
            sup = sp.tile([128, 128], f16, tag="sup")
            ident = sp.tile([128, 128], f16, tag="ident")
            psd = pp.tile([128, W], f32, tag="psd")
            psu = pp.tile([128, W], f32, tag="psu")
            psr = []
            for r in range(4):
                pst = pp.tile([128, W], f32, tag=f"psr{r}", name=f"psr{r}")
                psr.append(pst)

            xv0 = xb0[:].rearrange("p (s w) -> p s w", w=WP)
            xv1 = xb1[:].rearrange("p (s w) -> p s w", w=WP)
            xviews = [xv0, xv1]
            xtiles = [xb0, xb1]

            # ---------------- init / loads ----------------
            nc.vector.memset(xb0[:], 0.0)
            nc.gpsimd.memset(xb1[:], 0.0)
            nc.gpsimd.memset(ones[:], 1.0)

            nc.sync.dma_start(
                out=stage[:].rearrange("p (r j) -> p r j", j=W),
                in_=x_d.rearrange("(p r) w -> p r w", p=128),
            )
            nc.sync.dma_start(out=sdn[:], in_=sdn_d[:, :])
            nc.sync.dma_start(out=sup[:], in_=sup_d[:, :])
            nc.sync.dma_start(out=ident[:], in_=id_d[:, :])
            # w/b broadcast to all partitions via a K=1 matmul through PSUM
            nc.sync.dma_start(out=wbc[:1, :72], in_=w_d[None, :])
            nc.sync.dma_start(out=wbc[:1, 72:80], in_=b_d[None, :])
            pw = pp.tile([128, 80], f32, tag="wps")
            nc.tensor.matmul(pw[:, :], ones[:1, :], wbc[:1, :80],
                             start=True, stop=True)
            nc.vector.tensor_copy(out=wbc[:, :80], in_=pw[:, :])

            # fp32 -> fp16 state (scalar engine), then initial halo rows
            nc.scalar.copy(out=xv0[:, 1:5, 1:1 + W],
                           in_=stage[:].rearrange("p (r j) -> p r j", j=W))

            def pe_halo(nxt):
                # halo rows via PE partition shift (PSUM) + Act copy-back;
                # boundary partitions get exact zeros from the shift matrices
                nc.tensor.matmul(psd[:, :], sdn[:, :], nxt[:, 4, 1:513],
                                 start=True, stop=True)
                nc.tensor.matmul(psu[:, :], sup[:, :], nxt[:, 1, 1:513],
                                 start=True, stop=True)
                nc.scalar.copy(out=nxt[:, 0, 1:513], in_=psd[:, :])
                nc.scalar.copy(out=nxt[:, 5, 1:513], in_=psu[:, :])

            pe_halo(xv0)

            # ---------------- affinity conv ----------------
            # per channel: 9 tensor_scalar products (fp16 4x) into tmp stack,
            # then PE accumulate chains + Act copy-back (see docstring).
            affv = aff[:].rearrange("p (c rj) -> p c rj", c=9)
            tmps = [tmpA, tmpB]
            for m in range(channels):
                cp = PLANE_OF[m]
                tm = tmps[m % 2]
                tv = tm[:].rearrange("p (c r j) -> p c r j", c=9, j=W)
                for t in range(9):
                    a, b3 = divmod(t, 3)
                    xin = xv0[:, a:a + 4, b3:b3 + W]
                    wsc = wbc[:, 9 * m + t:9 * m + t + 1]
                    if t == 0:
                        nc.vector.tensor_scalar(
                            out=tv[:, 0], in0=xin, scalar1=wsc,
                            scalar2=wbc[:, 72 + m:73 + m], op0=mult, op1=add)
                    else:
                        nc.vector.tensor_scalar(
                            out=tv[:, t], in0=xin, scalar1=wsc, scalar2=None,
                            op0=mult)
                # 9-way sum on the tensor engine: chain of accumulating
                # matmuls with an identity stationary (exact fp32 in PSUM),
                # idle scalar engine copies back; next channel's products
                # overlap on DVE via the double-buffered scratch stack.
                avj = affv[:, cp].rearrange("p (r j) -> p r j", j=W)
                for r in range(4):
                    for c in range(9):
                        nc.tensor.matmul(psr[r][:, :], ident[:, :],
                                         tv[:, c, r, :],
                                         start=(c == 0), stop=(c == 8))
                    nc.scalar.copy(out=avj[:, r, :], in_=psr[r][:, :])

            # ---------------- kernel generation ----------------
            # abs of the 8 aff planes (skip 4) into tmpB planes 0..7 via
            # sign-bit clear (uint16 bitwise_and, ts 4x), then fp16
            # add-pyramid (column-split) -> s_abs in tmpB plane 0
            u16 = mybir.dt.uint16
            band = mybir.AluOpType.bitwise_and
            tb = tmpB[:].rearrange("p (c rj) -> p c rj", c=9)
            tbj = tmpB[:].rearrange("p (c r j) -> p c r j", c=9, j=W)
            tb_u = tmpB[:].bitcast(u16).rearrange("p (c rj) -> p c rj", c=9)
            aff_u = aff[:].bitcast(u16).rearrange("p (c rj) -> p c rj", c=9)
            nc.gpsimd.memset(absmask[:], 0x7FFF)
            for i, cp in enumerate([0, 1, 2, 3, 5, 6, 7, 8]):
                nc.vector.tensor_scalar(out=tb_u[:, i], in0=aff_u[:, cp],
                                        scalar1=absmask[:], scalar2=None, op0=band)
            nc.vector.tensor_tensor(out=tb[:, 0:4], in0=tb[:, 0:4],
                                    in1=tb[:, 4:8], op=add)
            nc.vector.tensor_tensor(out=tb[:, 0:2], in0=tb[:, 0:2],
                                    in1=tb[:, 2:4], op=add)
            nc.vector.tensor_tensor(out=tb[:, 0], in0=tb[:, 0],
                                    in1=tb[:, 1], op=add)
            # abs-sum (fp16) -> fp32, fast-NR reciprocal, halve+convert on Act
            nc.vector.tensor_copy(out=sums[:], in_=tb[:, 0])
            nc.vector.reciprocal_approx_fast(stage[:], sums[:])
            nc.scalar.mul(out=reciph[:], in_=stage[:], mul=0.5)
            # kern planes = aff planes * reciph (fp16 2x), split DVE/Pool
            kv = kern[:].rearrange("p (c rj) -> p c rj", c=9)
            kvj = kern[:].rearrange("p (c r j) -> p c r j", c=9, j=W)
            rb = reciph[:].unsqueeze(1).broadcast_to([128, 4, RJ])
            nc.vector.tensor_tensor(out=kv[:, 0:4], in0=affv[:, 0:4], in1=rb, op=mult)
            nc.vector.tensor_tensor(out=kv[:, 5:9], in0=affv[:, 5:9], in1=rb, op=mult)
            # s_half tree: T1 = K[0:4]+K[5:9]; T2 = T1[0:2]+T1[2:4]; s = T2[0]+T2[1]
            tv = tmpA[:].rearrange("p (c rj) -> p c rj", c=9)
            nc.vector.tensor_tensor(out=tv[:, 0:4], in0=kv[:, 0:4], in1=kv[:, 5:9], op=add)
            nc.vector.tensor_tensor(out=tv[:, 0:2], in0=tv[:, 0:2], in1=tv[:, 2:4], op=add)
            nc.vector.tensor_tensor(out=tv[:, 0], in0=tv[:, 0], in1=tv[:, 1], op=add)
            # kern plane 4 = 0.5 - s_half
            nc.vector.tensor_scalar(out=kv[:, 4], in0=tv[:, 0], scalar1=-1.0,
                                    scalar2=0.5, op0=mult, op1=add)

            # ---------------- diffusion ----------------
            # DVE computes only the 9 shifted products (fp16 2x mode, one op
            # per row x plane-group, halo-consuming rows last).  The 9-way
            # summation runs on the otherwise-idle tensor engine as chains of
            # accumulating identity matmuls into PSUM (one chain per image
            # row, exact fp32), and the scalar engine copies each row back
            # to the next-state buffer in fp16.  Halo rows then travel
            # across partitions via shifted-identity matmuls (pe_halo).
            prod = tmpA
            pv = prod[:].rearrange("p (c r j) -> p c r j", c=9, j=W)

            def product_row(curt, r, a):
                xap = curt[:]
                in1 = AP(xap.tensor, (a + r) * WP,
                         [list(xap.ap[0]), [1, 3], [WP, 1], [1, W]])
                nc.vector.tensor_tensor(out=pv[:, 3 * a:3 * a + 3, r:r + 1, :],
                                        in0=kvj[:, 3 * a:3 * a + 3, r:r + 1, :],
                                        in1=in1, op=mult)

            for it in range(iters):
                curt = xtiles[it % 2]
                nxt = xviews[(it + 1) % 2]
                for r in (1, 2, 0, 3):
                    for a in (1, 0, 2):
                        product_row(curt, r, a)
                for r in (1, 2, 0, 3):
                    for c in range(9):
                        nc.tensor.matmul(psr[r][:, :], ident[:, :],
                                         pv[:, c, r, :],
                                         start=(c == 0), stop=(c == 8))
                    nc.scalar.copy(out=nxt[:, 1 + r, 1:513], in_=psr[r][:, :])
                if it + 1 < iters:
                    pe_halo(nxt)

            # ---------------- output: fp16 -> fp32 * 2^24 ----------------
            nc.scalar.mul(out=stage[:].rearrange("p (r j) -> p r j", j=W),
                          in_=xviews[iters % 2][:, 1:5, 1:513], mul=float(2.0 ** 24))
            nc.sync.dma_start(
                out=out_d.rearrange("(p r) w -> p r w", p=128),
                in_=stage[:].rearrange("p (r j) -> p r j", j=W),
            )

    nc.finalize()
    return nc


def _get_program():
    global _PROGRAM
    if _PROGRAM is None:
        _PROGRAM = _build_program()
    return _PROGRAM


def kernel(x, W_aff, b_aff):
    from concourse.bass_utils import run_bass_kernel_spmd

    nc = _get_program()
    x = np.ascontiguousarray(np.asarray(x, dtype=np.float32))
    w = np.ascontiguousarray(np.asarray(W_aff, dtype=np.float32)).reshape(AFF_CH * 9)
    b = np.ascontiguousarray(np.asarray(b_aff, dtype=np.float32))
    sdn = np.zeros((128, 128), np.float16)
    sdn[np.arange(127), np.arange(1, 128)] = 1
    sup = np.zeros((128, 128), np.float16)
    sup[np.arange(1, 128), np.arange(127)] = 1
    ident = np.eye(128, dtype=np.float16)

    in_maps = [{"x": x[i, 0], "w_aff": w, "b_aff": b, "sdn": sdn, "sup": sup,
                "ident": ident} for i in range(B)]
    res = run_bass_kernel_spmd(nc, in_maps, list(range(B))).results
    out = np.stack([res[i]["out"] for i in range(B)], axis=0)[:, None]
    return out.astype(np.float32)


# revision 20
# speedup vs baseline: 4.4365x; 1.0975x over previous
"""CSPN (convolutional spatial propagation network) kernel for Trainium2.

Reference computation (per batch image, 512x512, fp32):
  aff    = conv3x3(x, W_aff, SAME) + b_aff          # 8 channels
  a      = aff / sum_c |aff_c| ; s = sum_c a_c
  kernel = concat([1 - s, a])                       # 9 channels
  24 iterations:  x <- sum_k kernel_k * shift_{OFFS[k]}(x)   (zero padded)

Sharding: data-parallel over batch, one image per NeuronCore (8 cores).

Per-core design (all SBUF resident, all four engines in play):
  * state in fp16; the 9-plane kernel is pre-scaled by 0.5 so every
    diffusion step halves the field (keeps fp16 in range); the final
    output is scaled back by 2^24 during the fp16->fp32 copy-out.
    Measured rel err ~9.4e-3 vs fp32 reference (2e-2 budget).
  * x state ping/pong [128 part, 6*514] fp16: partition p holds rows
    4p..4p+3 in slots 1..4, halo rows in slots 0/5, zero pad columns.
  * kernel planes [128, 9*2048] fp16, plane c=3a+b multiplies
    x[r+a-1, j+b-1] (ref channel (oi,oj) lands at plane (1-oi)*3+(1-oj)).
  * diffusion iteration (engines pipelined per image row):
      - DVE: only the 9 shifted products (fp16 2x_1p mode, one op per
        image row covering all 9 planes via overlapping access patterns;
        halo-consuming rows ordered last),
      - PE: the 9-way summation as chains of accumulating matmuls with
        an identity stationary into PSUM (one chain per row, exact fp32),
      - Act: PSUM -> fp16 next-state copy per row,
      - halo rows travel across partitions via shifted-identity matmuls
        (pe_halo) + Act copy-back; boundary partitions get exact zeros.
        Measured ~6x cheaper than partition-shifted SBUF->SBUF DMAs.
      - GPSIMD/Pool measured far below its cost model on sliced ops
        (~2.2us fixed per op), so it only does init memsets.
  * affinity conv: per channel, 9 tensor_scalar products (x * w + b,
    4x DVE mode, weights broadcast per-partition via a K=1 matmul
    through PSUM) into a double-buffered scratch stack; PE accumulate
    chains + Act copies produce the aff plane while DVE starts the next
    channel.
  * kernel generation: |aff| via sign-bit clear (uint16 bitcast AND),
    fp16 abs-sum pyramid, fast-NR reciprocal, halve+convert on the
    scalar engine, fp16 normalize, sum pyramid, plane 4 = 0.5 - s_half.
"""

import numpy as np

H = 512
W = 512
B = 8
ITER = 24
# itertools.product([0,1,-1], repeat=2) order (matches reference OFFS)
OFFS = [(i, j) for i in (0, 1, -1) for j in (0, 1, -1)]

WP = W + 2            # padded row width
NSLOT = 6             # row slots per partition (1 halo + 4 + 1 halo)
RJ = 4 * W            # 2048 elems per plane per partition
AFF_CH = 8


# ref aff channel m (kernel channel m+1, offset OFFS[m+1]) -> plane (1-oi)*3+(1-oj)
PLANE_OF = [(1 - oi) * 3 + (1 - oj) for (oi, oj) in OFFS[1:]]

_PROGRAM = None


def _build_program(iters=ITER, channels=AFF_CH):
    import concourse.mybir as mybir
    from concourse import bacc, tile
    from concourse.ap import AP

    f32 = mybir.dt.float32
    f16 = mybir.dt.float16
    mult = mybir.AluOpType.mult
    add = mybir.AluOpType.add
    Ax = mybir.AxisListType.X

    nc = bacc.Bacc("TRN2", target_bir_lowering=False, debug=False, name="cspn")

    x_d = nc.dram_tensor("x", [H, W], f32, kind="ExternalInput")
    sdn_d = nc.dram_tensor("sdn", [128, 128], f16, kind="ExternalInput")
    sup_d = nc.dram_tensor("sup", [128, 128], f16, kind="ExternalInput")
    id_d = nc.dram_tensor("ident", [128, 128], f16, kind="ExternalInput")
    w_d = nc.dram_tensor("w_aff", [AFF_CH * 9], f32, kind="ExternalInput")
    b_d = nc.dram_tensor("b_aff", [AFF_CH], f32, kind="ExternalInput")
    out_d = nc.dram_tensor("out", [H, W], f32, kind="ExternalOutput")

    with tile.TileContext(nc) as tc:
        with (
            nc.allow_low_precision(reason="fp16 scheme validated: rel err ~6e-3 vs 2e-2 budget"),
            tc.tile_pool(name="state", bufs=1) as sp,
            tc.tile_pool(name="psum", bufs=1, space="PSUM") as pp,
        ):
            xb0 = sp.tile([128, NSLOT * WP], f16, tag="xb0")
            xb1 = sp.tile([128, NSLOT * WP], f16, tag="xb1")
            kern = sp.tile([128, 9 * RJ], f16, tag="kern")
            tmpA = sp.tile([128, 9 * RJ], f16, tag="tmpA")
            tmpB = sp.tile([128, 9 * RJ], f16, tag="tmpB")
            aff = sp.tile([128, 9 * RJ], f16, tag="aff")  # 9 planes (4 unused)
            stage = sp.tile([128, RJ], f32, tag="stage")
            sums = sp.tile([128, RJ], f32, tag="sums")
            reciph = sp.tile([128, RJ], f16, tag="reciph")
            wbc = sp.tile([128, 80], f32, tag="wbc")
            ones = sp.tile([1, 128], f32, tag="ones")
            absmask = sp.tile([128, 1], mybir.dt.uint16, tag="absmask")
            sdn = sp.tile([128, 128], f16, tag="sdn")
            sup = sp.tile([128, 128], f16, tag="sup")
            ident = sp.tile([128, 128], f16, tag="ident")
            psd = pp.tile([128, W], f32, tag="psd")
            psu = pp.tile([128, W], f32, tag="psu")
            psr = []
            for r in range(4):
                pst = pp.tile([128, W], f32, tag=f"psr{r}", name=f"psr{r}")
                psr.append(pst)

            xv0 = xb0[:].rearrange("p (s w) -> p s w", w=WP)
            xv1 = xb1[:].rearrange("p (s w) -> p s w", w=WP)
            xviews = [xv0, xv1]
            xtiles = [xb0, xb1]

            # ---------------- init / loads ----------------
            nc.vector.memset(xb0[:], 0.0)
            nc.gpsimd.memset(xb1[:], 0.0)
            nc.gpsimd.memset(ones[:], 1.0)

            nc.sync.dma_start(
                out=stage[:].rearrange("p (r j) -> p r j", j=W),
                in_=x_d.rearrange("(p r) w -> p r w", p=128),
            )
            nc.sync.dma_start(out=sdn[:], in_=sdn_d[:, :])
            nc.sync.dma_start(out=sup[:], in_=sup_d[:, :])
            nc.sync.dma_start(out=ident[:], in_=id_d[:, :])
            # w/b broadcast to all partitions via a K=1 matmul through PSUM
            nc.sync.dma_start(out=wbc[:1, :72], in_=w_d[None, :])
            nc.sync.dma_start(out=wbc[:1, 72:80], in_=b_d[None, :])
            pw = pp.tile([128, 80], f32, tag="wps")
            nc.tensor.matmul(pw[:, :], ones[:1, :], wbc[:1, :80],
                             start=True, stop=True)
            nc.vector.tensor_copy(out=wbc[:, :80], in_=pw[:, :])

            # fp32 -> fp16 state (scalar engine), then initial halo rows
            nc.scalar.copy(out=xv0[:, 1:5, 1:1 + W],
                           in_=stage[:].rearrange("p (r j) -> p r j", j=W))

            def pe_halo(nxt):
                # halo rows via PE partition shift (PSUM) + Act copy-back;
                # boundary partitions get exact zeros from the shift matrices
                nc.tensor.matmul(psd[:, :], sdn[:, :], nxt[:, 4, 1:513],
                                 start=True, stop=True)
                nc.tensor.matmul(psu[:, :], sup[:, :], nxt[:, 1, 1:513],
                                 start=True, stop=True)
                nc.scalar.copy(out=nxt[:, 0, 1:513], in_=psd[:, :])
                nc.scalar.copy(out=nxt[:, 5, 1:513], in_=psu[:, :])

            pe_halo(xv0)

            # ---------------- affinity conv ----------------
            # per channel: 9 tensor_scalar products (fp16 4x) into tmp stack,
            # then PE accumulate chains + Act copy-back (see docstring).
            affv = aff[:].rearrange("p (c rj) -> p c rj", c=9)
            tmps = [tmpA, tmpB]
            for m in range(channels):
                cp = PLANE_OF[m]
                tm = tmps[m % 2]
                tv = tm[:].rearrange("p (c r j) -> p c r j", c=9, j=W)
                for t in range(9):
                    a, b3 = divmod(t, 3)
                    xin = xv0[:, a:a + 4, b3:b3 + W]
                    wsc = wbc[:, 9 * m + t:9 * m + t + 1]
                    if t == 0:
                        nc.vector.tensor_scalar(
                            out=tv[:, 0], in0=xin, scalar1=wsc,
                            scalar2=wbc[:, 72 + m:73 + m], op0=mult, op1=add)
                    else:
                        nc.vector.tensor_scalar(
                            out=tv[:, t], in0=xin, scalar1=wsc, scalar2=None,
                            op0=mult)
                # 9-way sum on the tensor engine: chain of accumulating
                # matmuls with an identity stationary (exact fp32 in PSUM),
                # idle scalar engine copies back; next channel's products
                # overlap on DVE via the double-buffered scratch stack.
                avj = affv[:, cp].rearrange("p (r j) -> p r j", j=W)
                for r in range(4):
                    for c in range(9):
                        nc.tensor.matmul(psr[r][:, :], ident[:, :],
                                         tv[:, c, r, :],
                                         start=(c == 0), stop=(c == 8))
                    nc.scalar.copy(out=avj[:, r, :], in_=psr[r][:, :])

            # ---------------- kernel generation ----------------
            # abs of the 8 aff planes (skip 4) into tmpB planes 0..7 via
            # sign-bit clear (uint16 bitwise_and, ts 4x), then fp16
            # add-pyramid (column-split) -> s_abs in tmpB plane 0
            u16 = mybir.dt.uint16
            band = mybir.AluOpType.bitwise_and
            tb = tmpB[:].rearrange("p (c rj) -> p c rj", c=9)
            tbj = tmpB[:].rearrange("p (c r j) -> p c r j", c=9, j=W)
            tb_u = tmpB[:].bitcast(u16).rearrange("p (c rj) -> p c rj", c=9)
            aff_u = aff[:].bitcast(u16).rearrange("p (c rj) -> p c rj", c=9)
            nc.gpsimd.memset(absmask[:], 0x7FFF)
            for i, cp in enumerate([0, 1, 2, 3, 5, 6, 7, 8]):
                nc.vector.tensor_scalar(out=tb_u[:, i], in0=aff_u[:, cp],
                                        scalar1=absmask[:], scalar2=None, op0=band)
            nc.vector.tensor_tensor(out=tb[:, 0:4], in0=tb[:, 0:4],
                                    in1=tb[:, 4:8], op=add)
            nc.vector.tensor_tensor(out=tb[:, 0:2], in0=tb[:, 0:2],
                                    in1=tb[:, 2:4], op=add)
            nc.vector.tensor_tensor(out=tb[:, 0], in0=tb[:, 0],
                                    in1=tb[:, 1], op=add)
            # abs-sum (fp16) -> fp32, fast-NR reciprocal, halve+convert on Act
            nc.vector.tensor_copy(out=sums[:], in_=tb[:, 0])
            nc.vector.reciprocal_approx_fast(stage[:], sums[:])
            nc.scalar.mul(out=reciph[:], in_=stage[:], mul=0.5)
            # kern planes = aff planes * reciph (fp16 2x), split DVE/Pool
            kv = kern[:].rearrange("p (c rj) -> p c rj", c=9)
            kvj = kern[:].rearrange("p (c r j) -> p c r j", c=9, j=W)
            rb = reciph[:].unsqueeze(1).broadcast_to([128, 4, RJ])
            nc.vector.tensor_tensor(out=kv[:, 0:4], in0=affv[:, 0:4], in1=rb, op=mult)
            nc.vector.tensor_tensor(out=kv[:, 5:9], in0=affv[:, 5:9], in1=rb, op=mult)
            # s_half tree: T1 = K[0:4]+K[5:9]; T2 = T1[0:2]+T1[2:4]; s = T2[0]+T2[1]
            tv = tmpA[:].rearrange("p (c rj) -> p c rj", c=9)
            nc.vector.tensor_tensor(out=tv[:, 0:4], in0=kv[:, 0:4], in1=kv[:, 5:9], op=add)
            nc.vector.tensor_tensor(out=tv[:, 0:2], in0=tv[:, 0:2], in1=tv[:, 2:4], op=add)
            nc.vector.tensor_tensor(out=tv[:, 0], in0=tv[:, 0], in1=tv[:, 1], op=add)
            # kern plane 4 = 0.5 - s_half
            nc.vector.tensor_scalar(out=kv[:, 4], in0=tv[:, 0], scalar1=-1.0,
                                    scalar2=0.5, op0=mult, op1=add)

            # ---------------- diffusion ----------------
            # DVE computes only the 9 shifted products (fp16 2x mode, one op
            # per row x plane-group, halo-consuming rows last).  The 9-way
            # summation runs on the otherwise-idle tensor engine as chains of
            # accumulating identity matmuls into PSUM (one chain per image
            # row, exact fp32), and the scalar engine copies each row back
            # to the next-state buffer in fp16.  Halo rows then travel
            # across partitions via shifted-identity matmuls (pe_halo).
            prod = tmpA
            pv = prod[:].rearrange("p (c r j) -> p c r j", c=9, j=W)

            def product_row(curt, r):
                # all 9 planes for row r in one op: in1 walks slot r+a,
                # col j+b via overlapping dims [a: step WP][b: step 1][j]
                xap = curt[:]
                in1 = AP(xap.tensor, r * WP,
                         [list(xap.ap[0]), [WP, 3], [1, 3], [1, W]])
                nc.vector.tensor_tensor(out=pv[:, 0:9, r, :],
                                        in0=kvj[:, 0:9, r, :],
                                        in1=in1, op=mult)

            for it in range(iters):
                curt = xtiles[it % 2]
                nxt = xviews[(it + 1) % 2]
                for r in (1, 2, 0, 3):
                    product_row(curt, r)
                for r in (1, 2, 0, 3):
                    for c in range(9):
                        nc.tensor.matmul(psr[r][:, :], ident[:, :],
                                         pv[:, c, r, :],
                                         start=(c == 0), stop=(c == 8))
                    nc.scalar.copy(out=nxt[:, 1 + r, 1:513], in_=psr[r][:, :])
                if it + 1 < iters:
                    pe_halo(nxt)

            # ---------------- output: fp16 -> fp32 * 2^24 ----------------
            nc.scalar.mul(out=stage[:].rearrange("p (r j) -> p r j", j=W),
                          in_=xviews[iters % 2][:, 1:5, 1:513], mul=float(2.0 ** 24))
            nc.sync.dma_start(
                out=out_d.rearrange("(p r) w -> p r w", p=128),
                in_=stage[:].rearrange("p (r j) -> p r j", j=W),
            )

    nc.finalize()
    return nc


def _get_program():
    global _PROGRAM
    if _PROGRAM is None:
        _PROGRAM = _build_program()
    return _PROGRAM


def kernel(x, W_aff, b_aff):
    from concourse.bass_utils import run_bass_kernel_spmd

    nc = _get_program()
    x = np.ascontiguousarray(np.asarray(x, dtype=np.float32))
    w = np.ascontiguousarray(np.asarray(W_aff, dtype=np.float32)).reshape(AFF_CH * 9)
    b = np.ascontiguousarray(np.asarray(b_aff, dtype=np.float32))
    sdn = np.zeros((128, 128), np.float16)
    sdn[np.arange(127), np.arange(1, 128)] = 1
    sup = np.zeros((128, 128), np.float16)
    sup[np.arange(1, 128), np.arange(127)] = 1
    ident = np.eye(128, dtype=np.float16)

    in_maps = [{"x": x[i, 0], "w_aff": w, "b_aff": b, "sdn": sdn, "sup": sup,
                "ident": ident} for i in range(B)]
    res = run_bass_kernel_spmd(nc, in_maps, list(range(B))).results
    out = np.stack([res[i]["out"] for i in range(B)], axis=0)[:, None]
    return out.astype(np.float32)


# revision 21
# speedup vs baseline: 4.6302x; 1.0437x over previous
"""CSPN (convolutional spatial propagation network) kernel for Trainium2.

Reference computation (per batch image, 512x512, fp32):
  aff    = conv3x3(x, W_aff, SAME) + b_aff          # 8 channels
  a      = aff / sum_c |aff_c| ; s = sum_c a_c
  kernel = concat([1 - s, a])                       # 9 channels
  24 iterations:  x <- sum_k kernel_k * shift_{OFFS[k]}(x)   (zero padded)

Sharding: data-parallel over batch, one image per NeuronCore (8 cores).

Per-core design (all SBUF resident, all four engines in play):
  * state in fp16; the 9-plane kernel is pre-scaled by 0.5 so every
    diffusion step halves the field (keeps fp16 in range); the final
    output is scaled back by 2^24 during the fp16->fp32 copy-out.
    Measured rel err ~9.4e-3 vs fp32 reference (2e-2 budget).
  * x state ping/pong [128 part, 6*514] fp16: partition p holds rows
    4p..4p+3 in slots 1..4, halo rows in slots 0/5, zero pad columns.
  * kernel planes [128, 9*2048] fp16, plane c=3a+b multiplies
    x[r+a-1, j+b-1] (ref channel (oi,oj) lands at plane (1-oi)*3+(1-oj)).
  * diffusion iteration (engines pipelined per image row):
      - DVE: only the 9 shifted products (fp16 2x_1p mode, one op per
        image row covering all 9 planes via overlapping access patterns;
        halo-consuming rows ordered last),
      - PE: the 9-way summation as chains of accumulating matmuls with
        an identity stationary into PSUM (one chain per row, exact fp32),
      - Act: PSUM -> fp16 next-state copy per row,
      - halo rows travel across partitions via shifted-identity matmuls
        (pe_halo) + Act copy-back; boundary partitions get exact zeros.
        Measured ~6x cheaper than partition-shifted SBUF->SBUF DMAs.
      - GPSIMD/Pool measured far below its cost model on sliced ops
        (~2.2us fixed per op), so it only does init memsets.
  * affinity conv: per channel, 9 tensor_scalar products (x * w + b,
    4x DVE mode, weights broadcast per-partition via a K=1 matmul
    through PSUM) into a double-buffered scratch stack; PE accumulate
    chains + Act copies produce the aff plane while DVE starts the next
    channel.
  * kernel generation: |aff| via sign-bit clear (uint16 bitcast AND),
    fp16 abs-sum pyramid, fast-NR reciprocal, halve+convert on the
    scalar engine, fp16 normalize, sum pyramid, plane 4 = 0.5 - s_half.
"""

import numpy as np

H = 512
W = 512
B = 8
ITER = 24
# itertools.product([0,1,-1], repeat=2) order (matches reference OFFS)
OFFS = [(i, j) for i in (0, 1, -1) for j in (0, 1, -1)]

WP = W + 2            # padded row width
NSLOT = 6             # row slots per partition (1 halo + 4 + 1 halo)
RJ = 4 * W            # 2048 elems per plane per partition
AFF_CH = 8


# ref aff channel m (kernel channel m+1, offset OFFS[m+1]) -> plane (1-oi)*3+(1-oj)
PLANE_OF = [(1 - oi) * 3 + (1 - oj) for (oi, oj) in OFFS[1:]]

_PROGRAM = None


def _build_program(iters=ITER, channels=AFF_CH):
    import concourse.mybir as mybir
    from concourse import bacc, tile
    from concourse.ap import AP

    f32 = mybir.dt.float32
    f16 = mybir.dt.float16
    mult = mybir.AluOpType.mult
    add = mybir.AluOpType.add
    Ax = mybir.AxisListType.X

    nc = bacc.Bacc("TRN2", target_bir_lowering=False, debug=False, name="cspn")

    x_d = nc.dram_tensor("x", [H, W], f32, kind="ExternalInput")
    sdn_d = nc.dram_tensor("sdn", [128, 128], f16, kind="ExternalInput")
    sup_d = nc.dram_tensor("sup", [128, 128], f16, kind="ExternalInput")
    id_d = nc.dram_tensor("ident", [128, 128], f16, kind="ExternalInput")
    w_d = nc.dram_tensor("w_aff", [AFF_CH * 9], f32, kind="ExternalInput")
    b_d = nc.dram_tensor("b_aff", [AFF_CH], f32, kind="ExternalInput")
    out_d = nc.dram_tensor("out", [H, W], f32, kind="ExternalOutput")

    with tile.TileContext(nc) as tc:
        with (
            nc.allow_low_precision(reason="fp16 scheme validated: rel err ~6e-3 vs 2e-2 budget"),
            tc.tile_pool(name="state", bufs=1) as sp,
            tc.tile_pool(name="psum", bufs=1, space="PSUM") as pp,
        ):
            xb0 = sp.tile([128, NSLOT * WP], f16, tag="xb0")
            xb1 = sp.tile([128, NSLOT * WP], f16, tag="xb1")
            kern = sp.tile([128, 9 * RJ], f16, tag="kern")
            tmpA = sp.tile([128, 9 * RJ], f16, tag="tmpA")
            tmpB = sp.tile([128, 9 * RJ], f16, tag="tmpB")
            aff = sp.tile([128, 9 * RJ], f16, tag="aff")  # 9 planes (4 unused)
            stage = sp.tile([128, RJ], f32, tag="stage")
            sums = sp.tile([128, RJ], f32, tag="sums")
            reciph = sp.tile([128, RJ], f16, tag="reciph")
            wbc = sp.tile([128, 80], f32, tag="wbc")
            ones = sp.tile([1, 128], f32, tag="ones")
            absmask = sp.tile([128, 1], mybir.dt.uint16, tag="absmask")
            sdn = sp.tile([128, 128], f16, tag="sdn")
            sup = sp.tile([128, 128], f16, tag="sup")
            ident = sp.tile([128, 128], f16, tag="ident")
            psd = pp.tile([128, W], f32, tag="psd")
            psu = pp.tile([128, W], f32, tag="psu")
            psr = []
            for r in range(4):
                pst = pp.tile([128, W], f32, tag=f"psr{r}", name=f"psr{r}")
                psr.append(pst)

            xv0 = xb0[:].rearrange("p (s w) -> p s w", w=WP)
            xv1 = xb1[:].rearrange("p (s w) -> p s w", w=WP)
            xviews = [xv0, xv1]
            xtiles = [xb0, xb1]

            # ---------------- init / loads ----------------
            nc.vector.memset(xb0[:], 0.0)
            nc.gpsimd.memset(xb1[:], 0.0)
            nc.gpsimd.memset(ones[:], 1.0)

            nc.sync.dma_start(
                out=stage[:].rearrange("p (r j) -> p r j", j=W),
                in_=x_d.rearrange("(p r) w -> p r w", p=128),
            )
            nc.sync.dma_start(out=sdn[:], in_=sdn_d[:, :])
            nc.sync.dma_start(out=sup[:], in_=sup_d[:, :])
            nc.sync.dma_start(out=ident[:], in_=id_d[:, :])
            # w/b broadcast to all partitions via a K=1 matmul through PSUM
            nc.sync.dma_start(out=wbc[:1, :72], in_=w_d[None, :])
            nc.sync.dma_start(out=wbc[:1, 72:80], in_=b_d[None, :])
            pw = pp.tile([128, 80], f32, tag="wps")
            nc.tensor.matmul(pw[:, :], ones[:1, :], wbc[:1, :80],
                             start=True, stop=True)
            nc.vector.tensor_copy(out=wbc[:, :80], in_=pw[:, :])

            # fp32 -> fp16 state (scalar engine), then initial halo rows
            nc.scalar.copy(out=xv0[:, 1:5, 1:1 + W],
                           in_=stage[:].rearrange("p (r j) -> p r j", j=W))

            def pe_halo(nxt):
                # halo rows via PE partition shift (PSUM) + Act copy-back;
                # boundary partitions get exact zeros from the shift matrices
                nc.tensor.matmul(psd[:, :], sdn[:, :], nxt[:, 4, 1:513],
                                 start=True, stop=True)
                nc.tensor.matmul(psu[:, :], sup[:, :], nxt[:, 1, 1:513],
                                 start=True, stop=True)
                nc.scalar.copy(out=nxt[:, 0, 1:513], in_=psd[:, :])
                nc.scalar.copy(out=nxt[:, 5, 1:513], in_=psu[:, :])

            pe_halo(xv0)

            # ---------------- affinity conv ----------------
            # per channel: 9 tensor_scalar products (fp16 4x) into tmp stack,
            # then PE accumulate chains + Act copy-back (see docstring).
            affv = aff[:].rearrange("p (c rj) -> p c rj", c=9)
            tmps = [tmpA, tmpB]
            for m in range(channels):
                cp = PLANE_OF[m]
                tm = tmps[m % 2]
                tv = tm[:].rearrange("p (c r j) -> p c r j", c=9, j=W)
                for t in range(9):
                    a, b3 = divmod(t, 3)
                    xin = xv0[:, a:a + 4, b3:b3 + W]
                    wsc = wbc[:, 9 * m + t:9 * m + t + 1]
                    if t == 0:
                        nc.vector.tensor_scalar(
                            out=tv[:, 0], in0=xin, scalar1=wsc,
                            scalar2=wbc[:, 72 + m:73 + m], op0=mult, op1=add)
                    else:
                        nc.vector.tensor_scalar(
                            out=tv[:, t], in0=xin, scalar1=wsc, scalar2=None,
                            op0=mult)
                # 9-way sum on the tensor engine: chain of accumulating
                # matmuls with an identity stationary (exact fp32 in PSUM),
                # idle scalar engine copies back; next channel's products
                # overlap on DVE via the double-buffered scratch stack.
                avj = affv[:, cp].rearrange("p (r j) -> p r j", j=W)
                for r in range(4):
                    for c in range(9):
                        nc.tensor.matmul(psr[r][:, :], ident[:, :],
                                         tv[:, c, r, :],
                                         start=(c == 0), stop=(c == 8))
                    nc.scalar.copy(out=avj[:, r, :], in_=psr[r][:, :])

            # ---------------- kernel generation ----------------
            # abs of the 8 aff planes (skip 4) into tmpB planes 0..7 via
            # sign-bit clear (uint16 bitwise_and, ts 4x), then fp16
            # add-pyramid (column-split) -> s_abs in tmpB plane 0
            u16 = mybir.dt.uint16
            band = mybir.AluOpType.bitwise_and
            tb = tmpB[:].rearrange("p (c rj) -> p c rj", c=9)
            tbj = tmpB[:].rearrange("p (c r j) -> p c r j", c=9, j=W)
            tb_u = tmpB[:].bitcast(u16).rearrange("p (c rj) -> p c rj", c=9)
            aff_u = aff[:].bitcast(u16).rearrange("p (c rj) -> p c rj", c=9)
            nc.gpsimd.memset(absmask[:], 0x7FFF)
            for i, cp in enumerate([0, 1, 2, 3, 5, 6, 7, 8]):
                nc.vector.tensor_scalar(out=tb_u[:, i], in0=aff_u[:, cp],
                                        scalar1=absmask[:], scalar2=None, op0=band)
            nc.vector.tensor_tensor(out=tb[:, 0:4], in0=tb[:, 0:4],
                                    in1=tb[:, 4:8], op=add)
            nc.vector.tensor_tensor(out=tb[:, 0:2], in0=tb[:, 0:2],
                                    in1=tb[:, 2:4], op=add)
            nc.vector.tensor_tensor(out=tb[:, 0], in0=tb[:, 0],
                                    in1=tb[:, 1], op=add)
            # abs-sum (fp16) -> fp32, fast-NR reciprocal, halve+convert on Act
            nc.vector.tensor_copy(out=sums[:], in_=tb[:, 0])
            nc.vector.reciprocal_approx_fast(stage[:], sums[:])
            nc.scalar.mul(out=reciph[:], in_=stage[:], mul=0.5)
            # kern planes = aff planes * reciph (fp16 2x), split DVE/Pool
            kv = kern[:].rearrange("p (c rj) -> p c rj", c=9)
            kvj = kern[:].rearrange("p (c r j) -> p c r j", c=9, j=W)
            rb = reciph[:].unsqueeze(1).broadcast_to([128, 4, RJ])
            nc.vector.tensor_tensor(out=kv[:, 0:4], in0=affv[:, 0:4], in1=rb, op=mult)
            nc.vector.tensor_tensor(out=kv[:, 5:9], in0=affv[:, 5:9], in1=rb, op=mult)
            # s_half tree: T1 = K[0:4]+K[5:9]; T2 = T1[0:2]+T1[2:4]; s = T2[0]+T2[1]
            tv = tmpA[:].rearrange("p (c rj) -> p c rj", c=9)
            nc.vector.tensor_tensor(out=tv[:, 0:4], in0=kv[:, 0:4], in1=kv[:, 5:9], op=add)
            nc.vector.tensor_tensor(out=tv[:, 0:2], in0=tv[:, 0:2], in1=tv[:, 2:4], op=add)
            nc.vector.tensor_tensor(out=tv[:, 0], in0=tv[:, 0], in1=tv[:, 1], op=add)
            # kern plane 4 = 0.5 - s_half
            nc.vector.tensor_scalar(out=kv[:, 4], in0=tv[:, 0], scalar1=-1.0,
                                    scalar2=0.5, op0=mult, op1=add)

            # ---------------- diffusion ----------------
            # DVE computes only the 9 shifted products (fp16 2x mode, one op
            # per row x plane-group, halo-consuming rows last).  The 9-way
            # summation runs on the otherwise-idle tensor engine as chains of
            # accumulating identity matmuls into PSUM (one chain per image
            # row, exact fp32), and the scalar engine copies each row back
            # to the next-state buffer in fp16.  Halo rows then travel
            # across partitions via shifted-identity matmuls (pe_halo).
            prod = tmpA
            pv = prod[:].rearrange("p (c r j) -> p c r j", c=9, j=W)

            def product_row(curt, r):
                # all 9 planes for row r in one op: in1 walks slot r+a,
                # col j+b via overlapping dims [a: step WP][b: step 1][j]
                xap = curt[:]
                in1 = AP(xap.tensor, r * WP,
                         [list(xap.ap[0]), [WP, 3], [1, 3], [1, W]])
                nc.vector.tensor_tensor(out=pv[:, 0:9, r, :],
                                        in0=kvj[:, 0:9, r, :],
                                        in1=in1, op=mult)

            def product_row_a(curt, r, a):
                xap = curt[:]
                in1 = AP(xap.tensor, (a + r) * WP,
                         [list(xap.ap[0]), [1, 3], [WP, 1], [1, W]])
                nc.vector.tensor_tensor(out=pv[:, 3 * a:3 * a + 3, r:r + 1, :],
                                        in0=kvj[:, 3 * a:3 * a + 3, r:r + 1, :],
                                        in1=in1, op=mult)

            for it in range(iters):
                curt = xtiles[it % 2]
                nxt = xviews[(it + 1) % 2]
                # row 1 split by shift so each piece waits on just one of
                # the previous iteration's copy-backs; other rows whole
                product_row_a(curt, 1, 1)
                product_row_a(curt, 1, 2)
                product_row_a(curt, 1, 0)
                for r in (2, 0, 3):
                    product_row(curt, r)
                for r in (1, 2, 0, 3):
                    for c in range(9):
                        nc.tensor.matmul(psr[r][:, :], ident[:, :],
                                         pv[:, c, r, :],
                                         start=(c == 0), stop=(c == 8))
                    nc.scalar.copy(out=nxt[:, 1 + r, 1:513], in_=psr[r][:, :])
                    if r == 0 and it + 1 < iters:
                        # up-halo needs only row 0: launch as soon as it lands
                        nc.tensor.matmul(psu[:, :], sup[:, :], nxt[:, 1, 1:513],
                                         start=True, stop=True)
                        nc.scalar.copy(out=nxt[:, 5, 1:513], in_=psu[:, :])
                if it + 1 < iters:
                    nc.tensor.matmul(psd[:, :], sdn[:, :], nxt[:, 4, 1:513],
                                     start=True, stop=True)
                    nc.scalar.copy(out=nxt[:, 0, 1:513], in_=psd[:, :])

            # ---------------- output: fp16 -> fp32 * 2^24 ----------------
            nc.scalar.mul(out=stage[:].rearrange("p (r j) -> p r j", j=W),
                          in_=xviews[iters % 2][:, 1:5, 1:513], mul=float(2.0 ** 24))
            nc.sync.dma_start(
                out=out_d.rearrange("(p r) w -> p r w", p=128),
                in_=stage[:].rearrange("p (r j) -> p r j", j=W),
            )

    nc.finalize()
    return nc


def _get_program():
    global _PROGRAM
    if _PROGRAM is None:
        _PROGRAM = _build_program()
    return _PROGRAM


def kernel(x, W_aff, b_aff):
    from concourse.bass_utils import run_bass_kernel_spmd

    nc = _get_program()
    x = np.ascontiguousarray(np.asarray(x, dtype=np.float32))
    w = np.ascontiguousarray(np.asarray(W_aff, dtype=np.float32)).reshape(AFF_CH * 9)
    b = np.ascontiguousarray(np.asarray(b_aff, dtype=np.float32))
    sdn = np.zeros((128, 128), np.float16)
    sdn[np.arange(127), np.arange(1, 128)] = 1
    sup = np.zeros((128, 128), np.float16)
    sup[np.arange(1, 128), np.arange(127)] = 1
    ident = np.eye(128, dtype=np.float16)

    in_maps = [{"x": x[i, 0], "w_aff": w, "b_aff": b, "sdn": sdn, "sup": sup,
                "ident": ident} for i in range(B)]
    res = run_bass_kernel_spmd(nc, in_maps, list(range(B))).results
    out = np.stack([res[i]["out"] for i in range(B)], axis=0)[:, None]
    return out.astype(np.float32)


# revision 23
# speedup vs baseline: 6.2734x; 1.3549x over previous
"""CSPN (convolutional spatial propagation network) kernel for Trainium2.

Reference computation (per batch image, 512x512, fp32):
  aff    = conv3x3(x, W_aff, SAME) + b_aff          # 8 channels
  a      = aff / sum_c |aff_c| ; s = sum_c a_c
  kernel = concat([1 - s, a])                       # 9 channels
  24 iterations:  x <- sum_k kernel_k * shift_{OFFS[k]}(x)   (zero padded)

Sharding: data-parallel over batch, one image per NeuronCore (8 cores).

Per-core design (all SBUF resident, all four engines in play):
  * state in fp16; the 9-plane kernel is pre-scaled by 0.5 so every
    diffusion step halves the field (keeps fp16 in range); the final
    output is scaled back by 2^24 during the fp16->fp32 copy-out.
    Measured rel err ~9.4e-3 vs fp32 reference (2e-2 budget).
  * x state ping/pong [128 part, 6*514] fp16: partition p holds rows
    4p..4p+3 in slots 1..4, halo rows in slots 0/5, zero pad columns.
  * kernel planes [128, 9*2048] fp16, plane c=3a+b multiplies
    x[r+a-1, j+b-1] (ref channel (oi,oj) lands at plane (1-oi)*3+(1-oj)).
  * diffusion iteration (engines pipelined per image row):
      - DVE: only the 9 shifted products (fp16 2x_1p mode, one op per
        image row covering all 9 planes via overlapping access patterns;
        halo-consuming rows ordered last),
      - PE: the 9-way summation as chains of accumulating matmuls with
        an identity stationary into PSUM (one chain per row, exact fp32),
      - Act: PSUM -> fp16 next-state copy per row,
      - halo rows travel across partitions via shifted-identity matmuls
        (pe_halo) + Act copy-back; boundary partitions get exact zeros.
        Measured ~6x cheaper than partition-shifted SBUF->SBUF DMAs.
      - GPSIMD/Pool measured far below its cost model on sliced ops
        (~2.2us fixed per op), so it only does init memsets.
  * affinity conv: per channel, 9 tensor_scalar products (x * w + b,
    4x DVE mode, weights broadcast per-partition via a K=1 matmul
    through PSUM) into a double-buffered scratch stack; PE accumulate
    chains + Act copies produce the aff plane while DVE starts the next
    channel.
  * kernel generation: |aff| via sign-bit clear (uint16 bitcast AND),
    fp16 abs-sum pyramid, fast-NR reciprocal, halve+convert on the
    scalar engine, fp16 normalize, sum pyramid, plane 4 = 0.5 - s_half.
"""

import numpy as np

H = 512
W = 512
B = 8
ITER = 24
# itertools.product([0,1,-1], repeat=2) order (matches reference OFFS)
OFFS = [(i, j) for i in (0, 1, -1) for j in (0, 1, -1)]

WP = W + 2            # padded row width
NSLOT = 6             # row slots per partition (1 halo + 4 + 1 halo)
RJ = 4 * W            # 2048 elems per plane per partition
AFF_CH = 8


# ref aff channel m (kernel channel m+1, offset OFFS[m+1]) -> plane (1-oi)*3+(1-oj)
PLANE_OF = [(1 - oi) * 3 + (1 - oj) for (oi, oj) in OFFS[1:]]

_PROGRAM = None


def _build_program(iters=ITER, channels=AFF_CH):
    import concourse.mybir as mybir
    from concourse import bacc, tile
    from concourse.ap import AP

    f32 = mybir.dt.float32
    f16 = mybir.dt.float16
    mult = mybir.AluOpType.mult
    add = mybir.AluOpType.add
    Ax = mybir.AxisListType.X

    nc = bacc.Bacc("TRN2", target_bir_lowering=False, debug=False, name="cspn")

    x_d = nc.dram_tensor("x", [H, W], f32, kind="ExternalInput")
    sdn_d = nc.dram_tensor("sdn", [128, 128], f16, kind="ExternalInput")
    sup_d = nc.dram_tensor("sup", [128, 128], f16, kind="ExternalInput")
    id_d = nc.dram_tensor("ident", [128, 128], f16, kind="ExternalInput")
    w_d = nc.dram_tensor("w_aff", [AFF_CH * 9], f32, kind="ExternalInput")
    b_d = nc.dram_tensor("b_aff", [AFF_CH], f32, kind="ExternalInput")
    out_d = nc.dram_tensor("out", [H, W], f32, kind="ExternalOutput")

    with tile.TileContext(nc) as tc:
        with (
            nc.allow_low_precision(reason="fp16 scheme validated: rel err ~6e-3 vs 2e-2 budget"),
            tc.tile_pool(name="state", bufs=1) as sp,
            tc.tile_pool(name="psum", bufs=1, space="PSUM") as pp,
        ):
            xb0 = sp.tile([128, NSLOT * WP], f16, tag="xb0")
            xb1 = sp.tile([128, NSLOT * WP], f16, tag="xb1")
            kern = sp.tile([128, 9 * RJ], f16, tag="kern")
            tmpA = sp.tile([128, 9 * RJ], f16, tag="tmpA")
            tmpB = sp.tile([128, 9 * RJ], f16, tag="tmpB")
            aff = sp.tile([128, 9 * RJ], f16, tag="aff")  # 9 planes (4 unused)
            stage = sp.tile([128, RJ], f32, tag="stage")
            sums = sp.tile([128, RJ], f32, tag="sums")
            reciph = sp.tile([128, RJ], f16, tag="reciph")
            wbc = sp.tile([128, 80], f32, tag="wbc")
            ones = sp.tile([1, 128], f32, tag="ones")
            absmask = sp.tile([128, 1], mybir.dt.uint16, tag="absmask")
            sdn = sp.tile([128, 128], f16, tag="sdn")
            sup = sp.tile([128, 128], f16, tag="sup")
            ident = sp.tile([128, 128], f16, tag="ident")
            psd = pp.tile([128, W], f32, tag="psd")
            psu = pp.tile([128, W], f32, tag="psu")
            psr = []
            for r in range(4):
                pst = pp.tile([128, W], f32, tag=f"psr{r}", name=f"psr{r}")
                psr.append(pst)

            xv0 = xb0[:].rearrange("p (s w) -> p s w", w=WP)
            xv1 = xb1[:].rearrange("p (s w) -> p s w", w=WP)
            xviews = [xv0, xv1]
            xtiles = [xb0, xb1]

            # ---------------- init / loads ----------------
            nc.vector.memset(xb0[:], 0.0)
            nc.gpsimd.memset(xb1[:], 0.0)
            nc.gpsimd.memset(ones[:], 1.0)

            nc.sync.dma_start(
                out=stage[:].rearrange("p (r j) -> p r j", j=W),
                in_=x_d.rearrange("(p r) w -> p r w", p=128),
            )
            nc.sync.dma_start(out=sdn[:], in_=sdn_d[:, :])
            nc.sync.dma_start(out=sup[:], in_=sup_d[:, :])
            nc.sync.dma_start(out=ident[:], in_=id_d[:, :])
            # w/b broadcast to all partitions via a K=1 matmul through PSUM
            nc.sync.dma_start(out=wbc[:1, :72], in_=w_d[None, :])
            nc.sync.dma_start(out=wbc[:1, 72:80], in_=b_d[None, :])
            pw = pp.tile([128, 80], f32, tag="wps")
            nc.tensor.matmul(pw[:, :], ones[:1, :], wbc[:1, :80],
                             start=True, stop=True)
            nc.vector.tensor_copy(out=wbc[:, :80], in_=pw[:, :])

            # fp32 -> fp16 state (scalar engine), then initial halo rows
            nc.scalar.copy(out=xv0[:, 1:5, 1:1 + W],
                           in_=stage[:].rearrange("p (r j) -> p r j", j=W))

            def pe_halo(nxt):
                # halo rows via PE partition shift (PSUM) + Act copy-back;
                # boundary partitions get exact zeros from the shift matrices
                nc.tensor.matmul(psd[:, :], sdn[:, :], nxt[:, 4, 1:513],
                                 start=True, stop=True)
                nc.tensor.matmul(psu[:, :], sup[:, :], nxt[:, 1, 1:513],
                                 start=True, stop=True)
                nc.scalar.copy(out=nxt[:, 0, 1:513], in_=psd[:, :])
                nc.scalar.copy(out=nxt[:, 5, 1:513], in_=psu[:, :])

            pe_halo(xv0)

            # ---------------- affinity conv ----------------
            # per channel: 9 tensor_scalar products (fp16 4x) into tmp stack,
            # then PE accumulate chains + Act copy-back (see docstring).
            affv = aff[:].rearrange("p (c rj) -> p c rj", c=9)
            tmps = [tmpA, tmpB]
            for m in range(channels):
                cp = PLANE_OF[m]
                tm = tmps[m % 2]
                tv = tm[:].rearrange("p (c r j) -> p c r j", c=9, j=W)
                for t in range(9):
                    a, b3 = divmod(t, 3)
                    xin = xv0[:, a:a + 4, b3:b3 + W]
                    wsc = wbc[:, 9 * m + t:9 * m + t + 1]
                    if t == 0:
                        nc.vector.tensor_scalar(
                            out=tv[:, 0], in0=xin, scalar1=wsc,
                            scalar2=wbc[:, 72 + m:73 + m], op0=mult, op1=add)
                    else:
                        nc.vector.tensor_scalar(
                            out=tv[:, t], in0=xin, scalar1=wsc, scalar2=None,
                            op0=mult)
                # 9-way sum on the tensor engine: chain of accumulating
                # matmuls with an identity stationary (exact fp32 in PSUM),
                # idle scalar engine copies back; next channel's products
                # overlap on DVE via the double-buffered scratch stack.
                avj = affv[:, cp].rearrange("p (r j) -> p r j", j=W)
                for r in range(4):
                    for c in range(9):
                        nc.tensor.matmul(psr[r][:, :], ident[:, :],
                                         tv[:, c, r, :],
                                         start=(c == 0), stop=(c == 8))
                    nc.scalar.copy(out=avj[:, r, :], in_=psr[r][:, :])

            # ---------------- kernel generation ----------------
            # abs of the 8 aff planes (skip 4) into tmpB planes 0..7 via
            # sign-bit clear (uint16 bitwise_and, ts 4x), then fp16
            # add-pyramid (column-split) -> s_abs in tmpB plane 0
            u16 = mybir.dt.uint16
            band = mybir.AluOpType.bitwise_and
            tb = tmpB[:].rearrange("p (c rj) -> p c rj", c=9)
            tbj = tmpB[:].rearrange("p (c r j) -> p c r j", c=9, j=W)
            tb_u = tmpB[:].bitcast(u16).rearrange("p (c rj) -> p c rj", c=9)
            aff_u = aff[:].bitcast(u16).rearrange("p (c rj) -> p c rj", c=9)
            nc.gpsimd.memset(absmask[:], 0x7FFF)
            for i, cp in enumerate([0, 1, 2, 3, 5, 6, 7, 8]):
                nc.vector.tensor_scalar(out=tb_u[:, i], in0=aff_u[:, cp],
                                        scalar1=absmask[:], scalar2=None, op0=band)
            nc.vector.tensor_tensor(out=tb[:, 0:4], in0=tb[:, 0:4],
                                    in1=tb[:, 4:8], op=add)
            nc.vector.tensor_tensor(out=tb[:, 0:2], in0=tb[:, 0:2],
                                    in1=tb[:, 2:4], op=add)
            nc.vector.tensor_tensor(out=tb[:, 0], in0=tb[:, 0],
                                    in1=tb[:, 1], op=add)
            # abs-sum (fp16) -> fp32, fast-NR reciprocal, halve+convert on Act
            nc.vector.tensor_copy(out=sums[:], in_=tb[:, 0])
            nc.vector.reciprocal_approx_fast(stage[:], sums[:])
            nc.scalar.mul(out=reciph[:], in_=stage[:], mul=0.5)
            # kern planes = aff planes * reciph (fp16 2x), split DVE/Pool
            kv = kern[:].rearrange("p (c rj) -> p c rj", c=9)
            kvj = kern[:].rearrange("p (c r j) -> p c r j", c=9, j=W)
            rb = reciph[:].unsqueeze(1).broadcast_to([128, 4, RJ])
            nc.vector.tensor_tensor(out=kv[:, 0:4], in0=affv[:, 0:4], in1=rb, op=mult)
            nc.vector.tensor_tensor(out=kv[:, 5:9], in0=affv[:, 5:9], in1=rb, op=mult)
            # s_half tree: T1 = K[0:4]+K[5:9]; T2 = T1[0:2]+T1[2:4]; s = T2[0]+T2[1]
            tv = tmpA[:].rearrange("p (c rj) -> p c rj", c=9)
            nc.vector.tensor_tensor(out=tv[:, 0:4], in0=kv[:, 0:4], in1=kv[:, 5:9], op=add)
            nc.vector.tensor_tensor(out=tv[:, 0:2], in0=tv[:, 0:2], in1=tv[:, 2:4], op=add)
            nc.vector.tensor_tensor(out=tv[:, 0], in0=tv[:, 0], in1=tv[:, 1], op=add)
            # kern plane 4 = 0.5 - s_half
            nc.vector.tensor_scalar(out=kv[:, 4], in0=tv[:, 0], scalar1=-1.0,
                                    scalar2=0.5, op0=mult, op1=add)

            # ---------------- diffusion ----------------
            # DVE computes only the 9 shifted products (fp16 2x mode, one op
            # per row x plane-group, halo-consuming rows last).  The 9-way
            # summation runs on the otherwise-idle tensor engine as chains of
            # accumulating identity matmuls into PSUM (one chain per image
            # row, exact fp32), and the scalar engine copies each row back
            # to the next-state buffer in fp16.  Halo rows then travel
            # across partitions via shifted-identity matmuls (pe_halo).
            prod = tmpA
            pv = prod[:].rearrange("p (c r j) -> p c r j", c=9, j=W)

            def product_row(curt, r):
                # all 9 planes for row r in one op: in1 walks slot r+a,
                # col j+b via overlapping dims [a: step WP][b: step 1][j]
                xap = curt[:]
                in1 = AP(xap.tensor, r * WP,
                         [list(xap.ap[0]), [WP, 3], [1, 3], [1, W]])
                nc.vector.tensor_tensor(out=pv[:, 0:9, r, :],
                                        in0=kvj[:, 0:9, r, :],
                                        in1=in1, op=mult)

            def product_row_a(curt, r, a):
                xap = curt[:]
                in1 = AP(xap.tensor, (a + r) * WP,
                         [list(xap.ap[0]), [1, 3], [WP, 1], [1, W]])
                nc.vector.tensor_tensor(out=pv[:, 3 * a:3 * a + 3, r:r + 1, :],
                                        in0=kvj[:, 3 * a:3 * a + 3, r:r + 1, :],
                                        in1=in1, op=mult)

            for it in range(iters):
                curt = xtiles[it % 2]
                nxt = xviews[(it + 1) % 2]
                # row 1 split by shift so each piece waits on just one of
                # the previous iteration's copy-backs; other rows whole
                product_row_a(curt, 1, 1)
                product_row_a(curt, 1, 2)
                product_row_a(curt, 1, 0)
                product_row(curt, 2)
                product_row_a(curt, 0, 1)
                product_row_a(curt, 0, 2)
                product_row_a(curt, 0, 0)   # down-halo piece last
                product_row(curt, 3)
                for r in (1, 2, 0, 3):
                    # row 1's chain consumes planes in piece-completion
                    # order so PE starts as soon as the first piece lands
                    order = (3, 4, 5, 6, 7, 8, 0, 1, 2) if r in (0, 1) \
                        else tuple(range(9))
                    for i_c, c in enumerate(order):
                        nc.tensor.matmul(psr[r][:, :], ident[:, :],
                                         pv[:, c, r, :],
                                         start=(i_c == 0), stop=(i_c == 8))
                    nc.scalar.copy(out=nxt[:, 1 + r, 1:513], in_=psr[r][:, :])
                    if r == 0 and it + 1 < iters:
                        # up-halo needs only row 0: launch as soon as it lands
                        nc.tensor.matmul(psu[:, :], sup[:, :], nxt[:, 1, 1:513],
                                         start=True, stop=True)
                        nc.scalar.copy(out=nxt[:, 5, 1:513], in_=psu[:, :])
                if it + 1 < iters:
                    nc.tensor.matmul(psd[:, :], sdn[:, :], nxt[:, 4, 1:513],
                                     start=True, stop=True)
                    nc.scalar.copy(out=nxt[:, 0, 1:513], in_=psd[:, :])

            # ---------------- output: fp16 -> fp32 * 2^24 ----------------
            nc.scalar.mul(out=stage[:].rearrange("p (r j) -> p r j", j=W),
                          in_=xviews[iters % 2][:, 1:5, 1:513], mul=float(2.0 ** 24))
            nc.sync.dma_start(
                out=out_d.rearrange("(p r) w -> p r w", p=128),
                in_=stage[:].rearrange("p (r j) -> p r j", j=W),
            )

    nc.finalize()
    return nc


def _get_program():
    global _PROGRAM
    if _PROGRAM is None:
        _PROGRAM = _build_program()
    return _PROGRAM


def kernel(x, W_aff, b_aff):
    from concourse.bass_utils import run_bass_kernel_spmd

    nc = _get_program()
    x = np.ascontiguousarray(np.asarray(x, dtype=np.float32))
    w = np.ascontiguousarray(np.asarray(W_aff, dtype=np.float32)).reshape(AFF_CH * 9)
    b = np.ascontiguousarray(np.asarray(b_aff, dtype=np.float32))
    sdn = np.zeros((128, 128), np.float16)
    sdn[np.arange(127), np.arange(1, 128)] = 1
    sup = np.zeros((128, 128), np.float16)
    sup[np.arange(1, 128), np.arange(127)] = 1
    ident = np.eye(128, dtype=np.float16)

    in_maps = [{"x": x[i, 0], "w_aff": w, "b_aff": b, "sdn": sdn, "sup": sup,
                "ident": ident} for i in range(B)]
    res = run_bass_kernel_spmd(nc, in_maps, list(range(B))).results
    out = np.stack([res[i]["out"] for i in range(B)], axis=0)[:, None]
    return out.astype(np.float32)
